# revision 7
# baseline (speedup 1.0000x reference)
"""ACE/SPADE block (nn_ACE_48808008352089) as a distributed Bass kernel on
8 TRN2 NeuronCores.

Sharding: data-parallel over (batch, image-half): core c handles batch
b = c // 2, rows [128*(c%2), 128*(c%2)+128).  BatchNorm is SyncBN via a
[128,2] AllReduce of per-core (sum, sumsq).  Small weights replicated.

Algorithm (per core, all label-dependent convs done as one-hot matmuls):
  - mu[j]   = relu(Wfc[j] @ style_codes[b,j] + bfc[j])          (TensorE)
  - A_g     = mu @ (ga * w_cg),  A_b = mu @ (ba * w_cb)         (TensorE)
  - S1/S2   = stacked shifted one-hot of labels (9 offsets x 19 labels
              as 114+57 partitions), built by broadcast-DMA + is_equal
  - actv    = relu(w_sh-conv(S) + b_sh)        (2 matmuls / row)
  - psum_g  = (1-ga)*w_g-conv(actv) + A_g-conv(S)   (11 MM / row-pair)
  - psum_b  = (1-ba)*w_b-conv(actv) + A_b-conv(S)
  - out     = norm * (psum_g + gbias) + (psum_b + bbias)
    where norm = (x + noise*nv - mean) * rstd  (global BN stats)
"""
import numpy as np
import ml_dtypes

from concourse import bacc, tile, mybir
from concourse.bass_utils import run_bass_kernel_spmd

BF16 = ml_dtypes.bfloat16

B, C, H, W, L, S = 4, 128, 256, 256, 19, 512
NCORES = 8
NROWS = 128                 # output rows per core
WP = W + 2                  # zero-padded width
NPIX = NROWS * W            # 32768
BLK = 8                     # output rows per block
NBLK = NROWS // BLK         # 16
QROWS = 32                  # rows per one-hot quarter
NQ = NROWS // QROWS         # 4
SQP = (QROWS + 2) * WP      # one-hot pixels per quarter (34*258)
LABN = (NROWS + 4) * WP + 2  # guarded labels buffer len (132*258+2)
OFFS9 = [(dy, dx) for dy in (-1, 0, 1) for dx in (-1, 0, 1)]
NTOT = float(B * H * W)
EPS = 1e-5

F32 = mybir.dt.float32
BF = mybir.dt.bfloat16
I8 = mybir.dt.int8
AL = mybir.AluOpType
AF = mybir.ActivationFunctionType

_cache = {}


def _s_rhs1(s3, r, q):
    """One-hot rhs for actv at actv-row r (relative to core): [*, 258]."""
    return s3[:, r - QROWS * q + 1, :]


def _s_rhs2(s3, r, q):
    """One-hot rhs for avg conv at out row pair (r, r+1): [*, 2, 256]."""
    lr = r - QROWS * q + 1
    return s3[:, lr:lr + 2, 1:W + 1]


def build_nc():
    nc = bacc.Bacc("TRN2", target_bir_lowering=False, debug=False,
                   num_devices=NCORES)
    d = {}
    def inp(name, shape, dt):
        d[name] = nc.dram_tensor(name, shape, dt, kind="ExternalInput")
        return d[name]

    x_d = inp("x", [C, NPIX], F32)
    noi_d = inp("noiseT", [NPIX], BF)
    lab_d = inp("labels", [LABN], I8)
    wfc_d = inp("wfc", [128, 304 * 128], BF)
    sct_d = inp("sct", [128, 4 * L], BF)
    bfct_d = inp("bfct", [128, 4 * L], F32)
    wcg_d = inp("wcg", [128, 4 * 1152], BF)
    wcb_d = inp("wcb", [128, 4 * 1152], BF)
    wsh1_d = inp("wsh1", [114, 128], BF)
    wsh2_d = inp("wsh2", [57, 128], BF)
    wg_d = inp("wg", [128, 9 * 128], BF)
    wb_d = inp("wb", [128, 9 * 128], BF)
    nv_d = inp("nv", [C, 1], F32)
    gbias_d = inp("gbias", [C, 1], F32)
    bbias_d = inp("bbias", [C, 1], F32)
    bshm_d = inp("bshm", [C, NROWS + 2], F32)   # b_sh masked per actv row
    jc114_d = inp("jc114", [114, 1], F32)
    jc57_d = inp("jc57", [57, 1], F32)
    out_d = nc.dram_tensor("out", [C, NPIX], F32, kind="ExternalOutput")
    import os
    DBG = bool(int(os.environ.get("KDBG", "0")))
    if DBG:
        dbg = {}
        for nm, shape, dt in [("d_stats", [C, 2], F32), ("d_rn", [C, 2], F32),
                              ("d_mu", [128, 4 * L], BF),
                              ("d_ag1", [114, 128], BF),
                              ("d_s1", [114, SQP], BF),
                              ("d_actv", [C, (BLK + 2) * WP], BF),
                              ("d_norm", [C, BLK * W], F32),
                              ("d_pg", [C, 2 * W], F32),
                              ("d_pb", [C, 2 * W], F32)]:
            dbg[nm] = nc.dram_tensor(nm, shape, dt, kind="ExternalOutput")

    with tile.TileContext(nc) as tc:
        with tc.tile_pool(name="const", bufs=1) as cp, \
             tc.tile_pool(name="dram", bufs=1, space="DRAM") as dramp:
            # ---- small constants ----
            nv = cp.tile([C, 1], F32)
            nc.sync.dma_start(out=nv[:], in_=nv_d[:])
            gbias = cp.tile([C, 1], F32)
            nc.sync.dma_start(out=gbias[:], in_=gbias_d[:])
            bbias = cp.tile([C, 1], F32)
            nc.sync.dma_start(out=bbias[:], in_=bbias_d[:])
            bshm = cp.tile([C, NROWS + 2], F32)
            nc.sync.dma_start(out=bshm[:], in_=bshm_d[:])
            jc114 = cp.tile([114, 1], F32)
            nc.sync.dma_start(out=jc114[:], in_=jc114_d[:])
            jc57 = cp.tile([57, 1], F32)
            nc.sync.dma_start(out=jc57[:], in_=jc57_d[:])
            wsh1 = cp.tile([114, 128], BF)
            nc.sync.dma_start(out=wsh1[:], in_=wsh1_d[:])
            wsh2 = cp.tile([57, 128], BF)
            nc.sync.dma_start(out=wsh2[:], in_=wsh2_d[:])
            wg = cp.tile([128, 9 * 128], BF)
            nc.sync.dma_start(out=wg[:], in_=wg_d[:])
            wb = cp.tile([128, 9 * 128], BF)
            nc.sync.dma_start(out=wb[:], in_=wb_d[:])
            wcg = cp.tile([128, 4 * 1152], BF)
            nc.sync.dma_start(out=wcg[:], in_=wcg_d[:])
            wcb = cp.tile([128, 4 * 1152], BF)
            nc.sync.dma_start(out=wcb[:], in_=wcb_d[:])
            ag1 = cp.tile([114, 128], BF)
            ag2 = cp.tile([57, 128], BF)
            ab1 = cp.tile([114, 128], BF)
            ab2 = cp.tile([57, 128], BF)
            sums_x = cp.tile([C, NBLK], F32)
            sums_q = cp.tile([C, NBLK], F32)
            stats2 = cp.tile([C, 2], F32)
            stats_g = cp.tile([C, 2], F32)
            m_t = cp.tile([C, 1], F32)
            e_t = cp.tile([C, 1], F32)
            nvar = cp.tile([C, 1], F32)
            varp = cp.tile([C, 1], F32)
            sqv = cp.tile([C, 1], F32)
            rstd = cp.tile([C, 1], F32)
            nmr = cp.tile([C, 1], F32)
            # persistent actv double-buffer (3D: [C, BLK+2, WP])
            actv_a = cp.tile([C, BLK + 2, WP], BF)
            actv_b = cp.tile([C, BLK + 2, WP], BF)
            actv_bufs = [actv_a, actv_b]
            # zero the padding columns once
            for ab in actv_bufs:
                nc.vector.memset(ab[:, :, 0:1], 0.0)
                nc.vector.memset(ab[:, :, WP - 1:WP], 0.0)
            # one-hot quarters (persistent, rebuilt per quarter;
            # compares on GpSimd so the DVE queue stays stats-free)
            s1 = cp.tile([114, QROWS + 2, WP], BF)
            s2 = cp.tile([57, QROWS + 2, WP], BF)

            def build_s(q):
                rep1 = rp.tile([114, QROWS + 2, WP], I8, tag="rep",
                               name=f"rep1_{q}")
                for g in range(6):
                    dy, dx = OFFS9[g]
                    base = (QROWS * q + 1 + dy) * WP + dx + 1
                    nc.scalar.dma_start(
                        out=rep1[g * L:(g + 1) * L, :, :],
                        in_=lab_d[base:base + SQP].partition_broadcast(L))
                nc.gpsimd.tensor_scalar(s1[:], rep1[:], jc114[:], None,
                                        AL.is_equal)
                rep2 = rp.tile([57, QROWS + 2, WP], I8, tag="rep",
                               name=f"rep2_{q}")
                for g in range(6, 9):
                    dy, dx = OFFS9[g]
                    base = (QROWS * q + 1 + dy) * WP + dx + 1
                    nc.scalar.dma_start(
                        out=rep2[(g - 6) * L:(g - 5) * L, :, :],
                        in_=lab_d[base:base + SQP].partition_broadcast(L))
                nc.gpsimd.tensor_scalar(s2[:], rep2[:], jc57[:], None,
                                        AL.is_equal)

            # ---- phase M: mu + A tables ----
            with tc.tile_pool(name="muws", bufs=2) as muws, \
                 tc.tile_pool(name="mu1", bufs=1) as mu1, \
                 tc.tile_pool(name="mups", bufs=1, space="PSUM") as mups:
                sct = mu1.tile([128, 4 * L], BF)
                nc.sync.dma_start(out=sct[:], in_=sct_d[:])
                bfct = mu1.tile([128, 4 * L], F32)
                nc.sync.dma_start(out=bfct[:], in_=bfct_d[:])
                mu_ps = [mups.tile([128, L], F32, tag=f"mups{m}", name=f"mups{m}")
                         for m in range(4)]
                # host layout: [dp, (dc, j, mc, o)] - each dc chunk contiguous
                wfc_sb = mu1.tile([128, 304 * 128], BF)
                CHK = 76 * 128
                for dc in range(4):
                    nc.sync.dma_start(out=wfc_sb[:, dc * CHK:(dc + 1) * CHK],
                                      in_=wfc_d[:, dc * CHK:(dc + 1) * CHK])
                for j in range(L):
                    for mc in range(4):
                        for dc in range(4):
                            nc.tensor.matmul(
                                mu_ps[mc][:, j:j + 1],
                                lhsT=wfc_sb[:, dc * CHK + (j * 4 + mc) * 128:
                                            dc * CHK + (j * 4 + mc + 1) * 128],
                                rhs=sct[:, dc * L + j:dc * L + j + 1],
                                start=(dc == 0), stop=(dc == 3))
                mu_sb = mu1.tile([128, 4 * L], BF)
                for mc in range(4):
                    mtmp = muws.tile([128, L], F32, tag="mtmp", name=f"mtmp{mc}")
                    nc.vector.tensor_add(mtmp[:], mu_ps[mc][:],
                                         bfct[:, mc * L:(mc + 1) * L])
                    nc.scalar.activation(mu_sb[:, mc * L:(mc + 1) * L], mtmp[:],
                                         AF.Relu)
                if DBG and mc == 3:
                    nc.sync.dma_start(out=dbg["d_mu"][:], in_=mu_sb[:])
                # A tables
                for tbl, (wsb, a1, a2) in enumerate(
                        [(wcg, ag1, ag2), (wcb, ab1, ab2)]):
                    for g in range(9):
                        aps = mups.tile([L, 128], F32, tag="aps", name=f"aps{tbl}{g}")
                        for mc in range(4):
                            nc.tensor.matmul(
                                aps[:],
                                lhsT=mu_sb[:, mc * L:(mc + 1) * L],
                                rhs=wsb[:, mc * 1152 + g * 128:mc * 1152 + (g + 1) * 128],
                                start=(mc == 0), stop=(mc == 3))
                        atmp = muws.tile([L, 128], BF, tag="atmp", name=f"atmp{tbl}{g}")
                        nc.scalar.copy(atmp[:], aps[:])
                        dst = (a1 if g < 6 else a2)
                        gg = g if g < 6 else g - 6
                        nc.sync.dma_start(out=dst[gg * L:(gg + 1) * L, :],
                                          in_=atmp[:])
                if DBG:
                    nc.sync.dma_start(out=dbg["d_ag1"][:], in_=ag1[:])

            # ---- main pools ----
            with tc.tile_pool(name="xp", bufs=2) as xp, \
                 tc.tile_pool(name="np_", bufs=2) as np_, \
                 tc.tile_pool(name="xnp", bufs=2) as xnp, \
                 tc.tile_pool(name="ob", bufs=2) as ob, \
                 tc.tile_pool(name="rp", bufs=2) as rp, \
                 tc.tile_pool(name="gsp", bufs=6) as gsp, \
                 tc.tile_pool(name="ps", bufs=2, space="PSUM") as psp, \
                 tc.tile_pool(name="psgb", bufs=4, space="PSUM") as psgb:

                build_s(0)
                # ---- phase A: BN partial sums ----
                for ci in range(NBLK):
                    xb = xp.tile([C, BLK * W], F32, tag="x", name=f"xA{ci}")
                    nc.sync.dma_start(out=xb[:],
                                      in_=x_d[:, ci * BLK * W:(ci + 1) * BLK * W])
                    nb = np_.tile([C, BLK * W], BF, tag="n", name=f"nA{ci}")
                    nc.sync.dma_start(
                        out=nb[:],
                        in_=noi_d[ci * BLK * W:(ci + 1) * BLK * W]
                        .partition_broadcast(C))
                    xnb = xnp.tile([C, BLK * W], F32, tag="xn", name=f"xnA{ci}")
                    nc.vector.scalar_tensor_tensor(
                        out=xnb[:], in0=nb[:], scalar=nv[:], in1=xb[:],
                        op0=AL.mult, op1=AL.add,
                        accum_out=sums_x[:, ci:ci + 1])
                    nc.scalar.activation(xnb[:], xnb[:], AF.Square,
                                         accum_out=sums_q[:, ci:ci + 1])
                nc.vector.tensor_reduce(out=stats2[:, 0:1], in_=sums_x[:],
                                        axis=mybir.AxisListType.X, op=AL.add)
                nc.vector.tensor_reduce(out=stats2[:, 1:2], in_=sums_q[:],
                                        axis=mybir.AxisListType.X, op=AL.add)
                stat_in = dramp.tile([C, 2], F32)
                stat_out = dramp.tile([C, 2], F32)
                nc.sync.dma_start(out=stat_in[:], in_=stats2[:])
                nc.gpsimd.collective_compute(
                    "AllReduce", AL.add, replica_groups=[list(range(NCORES))],
                    ins=[stat_in.opt()], outs=[stat_out.opt()])
                nc.sync.dma_start(out=stats_g[:], in_=stat_out[:])
                nc.vector.tensor_scalar_mul(m_t[:], stats_g[:, 0:1], 1.0 / NTOT)
                nc.vector.tensor_scalar_mul(e_t[:], stats_g[:, 1:2], 1.0 / NTOT)
                # nvar = m^2 - E[x^2];  varp = -nvar + EPS = var + EPS
                nc.vector.scalar_tensor_tensor(
                    out=nvar[:], in0=m_t[:], scalar=m_t[:], in1=e_t[:],
                    op0=AL.mult, op1=AL.subtract)
                nc.vector.tensor_scalar(varp[:], nvar[:], -1.0, EPS,
                                        AL.mult, AL.add)
                nc.scalar.activation(sqv[:], varp[:], AF.Sqrt)
                nc.vector.reciprocal(rstd[:], sqv[:])
                nc.vector.scalar_tensor_tensor(
                    out=nmr[:], in0=m_t[:], scalar=-1.0, in1=rstd[:],
                    op0=AL.mult, op1=AL.mult)
                if DBG:
                    nc.sync.dma_start(out=dbg["d_stats"][:], in_=stats_g[:])
                    rn = cp.tile([C, 2], F32)
                    nc.vector.tensor_copy(rn[:, 0:1], rstd[:])
                    nc.vector.tensor_copy(rn[:, 1:2], nmr[:])
                    nc.sync.dma_start(out=dbg["d_rn"][:], in_=rn[:])

                # ---- main pass ----
                for kb in range(NBLK):
                    q = kb // (QROWS // BLK)
                    r0 = kb * BLK
                    if kb % (QROWS // BLK) == 0 and kb > 0:
                        build_s(q)
                        if DBG and q == 0:
                            nc.sync.dma_start(out=dbg["d_s1"][:],
                                              in_=s1[:].rearrange("p a b -> p (a b)"))

                    xb = xp.tile([C, BLK * W], F32, tag="x", name=f"xM{kb}")
                    nc.sync.dma_start(out=xb[:],
                                      in_=x_d[:, r0 * W:(r0 + BLK) * W])
                    nb = np_.tile([C, BLK * W], BF, tag="n", name=f"nM{kb}")
                    nc.sync.dma_start(
                        out=nb[:],
                        in_=noi_d[r0 * W:(r0 + BLK) * W].partition_broadcast(C))
                    xnb = xnp.tile([C, BLK * W], F32, tag="xn", name=f"xnM{kb}")
                    nc.vector.scalar_tensor_tensor(
                        out=xnb[:], in0=nb[:], scalar=nv[:], in1=xb[:],
                        op0=AL.mult, op1=AL.add)
                    # norm in place on the DVE (ACT queue stays stats-free)
                    nc.vector.tensor_scalar(xnb[:], xnb[:], rstd[:], nmr[:],
                                            AL.mult, AL.add)
                    norm = xnb

                    # actv rows ar in [r0-1, r0+BLK+1)
                    actv = actv_bufs[kb % 2]
                    for ir in range(BLK + 2):
                        ar = r0 - 1 + ir
                        pa = psp.tile([C, WP], F32, tag="pa", name=f"pa{kb}_{ir}")
                        nc.tensor.matmul(pa[:], lhsT=wsh1[:],
                                         rhs=_s_rhs1(s1, ar, q),
                                         start=True, stop=False)
                        nc.tensor.matmul(pa[:], lhsT=wsh2[:],
                                         rhs=_s_rhs1(s2, ar, q),
                                         start=False, stop=True)
                        nc.scalar.activation(actv[:, ir, 1:W + 1],
                                             pa[:, 1:W + 1], AF.Relu,
                                             bias=bshm[:, ar + 1:ar + 2])

                    if DBG and kb == 0:
                        nc.sync.dma_start(out=dbg["d_actv"][:],
                                          in_=actv[:].rearrange("p a b -> p (a b)"))
                        nc.sync.dma_start(out=dbg["d_norm"][:], in_=norm[:])
                    # gamma/beta: weight-stationary over the block's 4 row
                    # pairs (1 LDWEIGHTS per 4 matmuls), staged to SBUF bf16
                    # so PE never waits on the BN stats / epilogue.
                    NP2 = BLK // 2
                    gstage = gsp.tile([C, BLK * W], BF, tag="gs", name=f"gs{kb}")
                    bstage = gsp.tile([C, BLK * W], BF, tag="bs", name=f"bs{kb}")
                    for pst_w, a1t, a2t, stage in ((wg, ag1, ag2, gstage),
                                                   (wb, ab1, ab2, bstage)):
                        pps = [psgb.tile([C, 2 * W], F32, tag="pgb",
                                        name=f"p{kb}_{id(stage) % 97}_{i}")
                               for i in range(NP2)]
                        for g, (dy, dx) in enumerate(OFFS9):
                            for i in range(NP2):
                                nc.tensor.matmul(
                                    pps[i][:],
                                    lhsT=pst_w[:, g * 128:(g + 1) * 128],
                                    rhs=actv[:, 2 * i + 1 + dy:2 * i + 3 + dy,
                                             1 + dx:W + 1 + dx],
                                    start=(g == 0), stop=False)
                        for i in range(NP2):
                            nc.tensor.matmul(pps[i][:], lhsT=a1t[:],
                                             rhs=_s_rhs2(s1, r0 + 2 * i, q),
                                             start=False, stop=False)
                        for i in range(NP2):
                            nc.tensor.matmul(pps[i][:], lhsT=a2t[:],
                                             rhs=_s_rhs2(s2, r0 + 2 * i, q),
                                             start=False, stop=True)
                        for i in range(NP2):
                            nc.scalar.copy(stage[:, 2 * i * W:(2 * i + 2) * W],
                                           pps[i][:])
                    if DBG and kb == 0:
                        nc.sync.dma_start(out=dbg["d_pg"][:],
                                          in_=gstage[:, 0:2 * W])
                        nc.sync.dma_start(out=dbg["d_pb"][:],
                                          in_=bstage[:, 0:2 * W])
                    ot = ob.tile([C, BLK * W], F32, tag="ot", name=f"ot{kb}")
                    nc.vector.scalar_tensor_tensor(
                        out=ot[:], in0=gstage[:], scalar=gbias[:],
                        in1=norm[:], op0=AL.add, op1=AL.mult)
                    nc.vector.scalar_tensor_tensor(
                        out=ot[:], in0=bstage[:], scalar=bbias[:], in1=ot[:],
                        op0=AL.add, op1=AL.add)
                    nc.sync.dma_start(out=out_d[:, r0 * W:(r0 + BLK) * W],
                                       in_=ot[:])
    nc.compile()
    return nc


def _prep_shared(inputs):
    """Host-side weight layout prep (replicated to all cores)."""
    gb = np.asarray(inputs["g_blend"], np.float32).reshape(-1)[0]
    bb = np.asarray(inputs["b_blend"], np.float32).reshape(-1)[0]
    ga = 1.0 / (1.0 + np.exp(-gb))
    ba = 1.0 / (1.0 + np.exp(-bb))
    w_sh = np.asarray(inputs["w_sh"], np.float32)
    w_g = np.asarray(inputs["w_g"], np.float32)
    w_b = np.asarray(inputs["w_b"], np.float32)
    w_cg = np.asarray(inputs["w_cg"], np.float32)
    w_cb = np.asarray(inputs["w_cb"], np.float32)
    Wfc = np.asarray(inputs["Wfc"], np.float32)
    bfc = np.asarray(inputs["bfc"], np.float32)
    b_sh = np.asarray(inputs["b_sh"], np.float32)
    b_g = np.asarray(inputs["b_g"], np.float32)
    b_b = np.asarray(inputs["b_b"], np.float32)
    b_cg = np.asarray(inputs["b_cg"], np.float32)
    b_cb = np.asarray(inputs["b_cb"], np.float32)
    nv = np.asarray(inputs["noise_var"], np.float32)

    sh = {}
    # w_sh [o, j, 3, 3] -> [(dy,dx,j), o] stacked
    wshst = np.ascontiguousarray(
        w_sh.transpose(2, 3, 1, 0).reshape(9 * L, 128)).astype(BF16)
    sh["wsh1"] = np.ascontiguousarray(wshst[:6 * L])
    sh["wsh2"] = np.ascontiguousarray(wshst[6 * L:])
    # w_g/w_b [o, c, 3, 3] -> [c, (g, o)] scaled
    sh["wg"] = np.ascontiguousarray(
        ((1 - ga) * w_g).transpose(1, 2, 3, 0).reshape(128, 9 * 128)).astype(BF16)
    sh["wb"] = np.ascontiguousarray(
        ((1 - ba) * w_b).transpose(1, 2, 3, 0).reshape(128, 9 * 128)).astype(BF16)
    # w_cg/w_cb [o, c(512), 3, 3] -> [128, (cc, g, o)] scaled
    def cvt_cw(wt, scale):
        a = (scale * wt).transpose(1, 2, 3, 0).reshape(512, 9 * 128)
        a = a.reshape(4, 128, 9 * 128).transpose(1, 0, 2).reshape(128, 4 * 1152)
        return np.ascontiguousarray(a).astype(BF16)
    sh["wcg"] = cvt_cw(w_cg, ga)
    sh["wcb"] = cvt_cw(w_cb, ba)
    # Wfc [j, o, d] -> [128(dp), (j, dc, mc)*128 + o]
    f = Wfc.transpose(0, 2, 1).reshape(L, 4, 128, 4, 128)
    f = f.transpose(0, 1, 3, 2, 4)            # [j, dc, mc, dp, o]
    f = f.transpose(3, 1, 0, 2, 4).reshape(128, 304 * 128)  # [dp, dc, j, mc, o]
    sh["wfc"] = np.ascontiguousarray(f).astype(BF16)
    # bfc [j, d] -> bfct [128, (mc, j)]
    bf_t = bfc.T.reshape(4, 128, L).transpose(1, 0, 2).reshape(128, 4 * L)
    sh["bfct"] = np.ascontiguousarray(bf_t).astype(np.float32)
    sh["nv"] = np.ascontiguousarray(nv.reshape(C, 1))
    sh["gbias"] = np.ascontiguousarray(
        (1.0 + ga * b_cg + (1 - ga) * b_g).reshape(C, 1)).astype(np.float32)
    sh["bbias"] = np.ascontiguousarray(
        (ba * b_cb + (1 - ba) * b_b).reshape(C, 1)).astype(np.float32)
    sh["jc114"] = np.tile(np.arange(L, dtype=np.float32), 6)[:, None].copy()
    sh["jc57"] = np.tile(np.arange(L, dtype=np.float32), 3)[:, None].copy()
    sh["_b_sh"] = b_sh
    return sh


def kernel(**inputs):
    if "nc" not in _cache:
        _cache["nc"] = build_nc()
    nc = _cache["nc"]

    x = np.asarray(inputs["x"], np.float32)
    labels = np.asarray(inputs["labels"]).astype(np.int64)
    noise = np.asarray(inputs["noise"], np.float32)
    style = np.asarray(inputs["style_codes"], np.float32)
    sh = _prep_shared(inputs)
    b_sh = sh.pop("_b_sh")

    in_maps = []
    for c in range(NCORES):
        b, half = c // 2, c % 2
        h0 = half * NROWS
        m = dict(sh)
        m["x"] = np.ascontiguousarray(
            x[b, :, h0:h0 + NROWS, :]).reshape(C, NPIX)
        # noise [B, W, H, 1]: added[c,h,w] = noise[b,w,h]*nv[c]
        m["noiseT"] = np.ascontiguousarray(
            noise[b, :, h0:h0 + NROWS, 0].T).reshape(NPIX).astype(BF16)
        # guarded, padded labels (int8, -1 outside image)
        lab = np.full((NROWS + 4, WP), -1, np.int8)
        lo, hi = max(0, h0 - 2), min(H, h0 + NROWS + 2)
        lab[lo - (h0 - 2):hi - (h0 - 2), 1:W + 1] = labels[b, lo:hi, :]
        g = np.full(LABN, -1, np.int8)
        g[1:1 + (NROWS + 4) * WP] = lab.reshape(-1)
        m["labels"] = g
        # style codes transposed [128, (dc, j)]
        sct = style[b].T.reshape(4, 128, L).transpose(1, 0, 2).reshape(128, 4 * L)
        m["sct"] = np.ascontiguousarray(sct).astype(BF16)
        # b_sh masked per actv row (zero outside image)
        rows = h0 + np.arange(-1, NROWS + 1)
        mask = ((rows >= 0) & (rows < H)).astype(np.float32)
        m["bshm"] = np.ascontiguousarray(b_sh[:, None] * mask[None, :])
        in_maps.append(m)

    res = run_bass_kernel_spmd(nc, in_maps, core_ids=list(range(NCORES)),
                               **_cache.get("run_kwargs", {}))
    _cache["last_result"] = res

    out = np.empty((B, C, H, W), np.float32)
    for c in range(NCORES):
        b, half = c // 2, c % 2
        h0 = half * NROWS
        out[b, :, h0:h0 + NROWS, :] = res.results[c]["out"].reshape(C, NROWS, W)
    return out


# revision 9
# speedup vs baseline: 1.9651x; 1.9651x over previous
"""ACE/SPADE block (nn_ACE_48808008352089) as a distributed Bass kernel on
8 TRN2 NeuronCores.

Sharding: data-parallel over (batch, image-half): core c handles batch
b = c // 2, rows [128*(c%2), 128*(c%2)+128).  BatchNorm is SyncBN via a
[128,2] AllReduce of per-core (sum, sumsq).  Small weights replicated.

Algorithm (per core, all label-dependent convs done as one-hot matmuls):
  - mu[j]   = relu(Wfc[j] @ style_codes[b,j] + bfc[j])          (TensorE)
  - A_g     = mu @ (ga * w_cg),  A_b = mu @ (ba * w_cb)         (TensorE)
  - S1/S2   = stacked shifted one-hot of labels (9 offsets x 19 labels
              as 114+57 partitions), built by broadcast-DMA + is_equal
  - actv    = relu(w_sh-conv(S) + b_sh)        (2 matmuls / row)
  - psum_g  = (1-ga)*w_g-conv(actv) + A_g-conv(S)   (11 MM / row-pair)
  - psum_b  = (1-ba)*w_b-conv(actv) + A_b-conv(S)
  - out     = norm * (psum_g + gbias) + (psum_b + bbias)
    where norm = (x + noise*nv - mean) * rstd  (global BN stats)
"""
import numpy as np
import ml_dtypes

from concourse import bacc, tile, mybir
from concourse.bass_utils import run_bass_kernel_spmd

BF16 = ml_dtypes.bfloat16

B, C, H, W, L, S = 4, 128, 256, 256, 19, 512
NCORES = 8
NROWS = 128                 # output rows per core
WP = W + 2                  # zero-padded width
NPIX = NROWS * W            # 32768
BLK = 8                     # output rows per block
NBLK = NROWS // BLK         # 16
QROWS = 32                  # rows per one-hot quarter
NQ = NROWS // QROWS         # 4
SQP = (QROWS + 2) * WP      # one-hot pixels per quarter (34*258)
LABN = (NROWS + 4) * WP + 2  # guarded labels buffer len (132*258+2)
OFFS9 = [(dy, dx) for dy in (-1, 0, 1) for dx in (-1, 0, 1)]
NTOT = float(B * H * W)
EPS = 1e-5

F32 = mybir.dt.float32
BF = mybir.dt.bfloat16
I8 = mybir.dt.int8
AL = mybir.AluOpType
AF = mybir.ActivationFunctionType

_cache = {}


def _s_rhs1(s3, r, q):
    """One-hot rhs for actv at actv-row r (relative to core): [*, 258]."""
    return s3[:, r - QROWS * q + 1, :]


def _s_rhs2(s3, r, q):
    """One-hot rhs for avg conv at out row pair (r, r+1): [*, 2, 256]."""
    lr = r - QROWS * q + 1
    return s3[:, lr:lr + 2, 1:W + 1]


def build_nc():
    nc = bacc.Bacc("TRN2", target_bir_lowering=False, debug=False,
                   num_devices=NCORES)
    d = {}
    def inp(name, shape, dt):
        d[name] = nc.dram_tensor(name, shape, dt, kind="ExternalInput")
        return d[name]

    x_d = inp("x", [C, NPIX], F32)
    noi_d = inp("noiseT", [NPIX], BF)
    lab_d = inp("labels", [LABN], I8)
    wfc_d = inp("wfc", [128, 304 * 128], BF)
    sct_d = inp("sct", [128, 4 * L], BF)
    bfct_d = inp("bfct", [128, 4 * L], F32)
    wcg_d = inp("wcg", [128, 4 * 1152], BF)
    wcb_d = inp("wcb", [128, 4 * 1152], BF)
    wsh1_d = inp("wsh1", [114, 128], BF)
    wsh2_d = inp("wsh2", [57, 128], BF)
    wg_d = inp("wg", [128, 9 * 128], BF)
    wb_d = inp("wb", [128, 9 * 128], BF)
    nv_d = inp("nv", [C, 1], F32)
    gbias_d = inp("gbias", [C, 1], F32)
    bbias_d = inp("bbias", [C, 1], F32)
    bshm_d = inp("bshm", [C, NROWS + 2], F32)   # b_sh masked per actv row
    jc114_d = inp("jc114", [114, 1], F32)
    jc57_d = inp("jc57", [57, 1], F32)
    out_d = nc.dram_tensor("out", [C, NPIX], F32, kind="ExternalOutput")
    import os
    DBG = bool(int(os.environ.get("KDBG", "0")))
    if DBG:
        dbg = {}
        for nm, shape, dt in [("d_stats", [C, 2], F32), ("d_rn", [C, 2], F32),
                              ("d_mu", [128, 4 * L], BF),
                              ("d_ag1", [114, 128], BF),
                              ("d_s1", [114, SQP], BF),
                              ("d_actv", [C, (BLK + 2) * WP], BF),
                              ("d_norm", [C, BLK * W], F32),
                              ("d_pg", [C, 2 * W], F32),
                              ("d_pb", [C, 2 * W], F32)]:
            dbg[nm] = nc.dram_tensor(nm, shape, dt, kind="ExternalOutput")

    with tile.TileContext(nc) as tc:
        with tc.tile_pool(name="const", bufs=1) as cp, \
             tc.tile_pool(name="dram", bufs=1, space="DRAM") as dramp:
            # ---- small constants ----
            nv = cp.tile([C, 1], F32)
            nc.sync.dma_start(out=nv[:], in_=nv_d[:])
            gbias = cp.tile([C, 1], F32)
            nc.sync.dma_start(out=gbias[:], in_=gbias_d[:])
            bbias = cp.tile([C, 1], F32)
            nc.sync.dma_start(out=bbias[:], in_=bbias_d[:])
            bshm = cp.tile([C, NROWS + 2], F32)
            nc.sync.dma_start(out=bshm[:], in_=bshm_d[:])
            jc114 = cp.tile([114, 1], F32)
            nc.sync.dma_start(out=jc114[:], in_=jc114_d[:])
            jc57 = cp.tile([57, 1], F32)
            nc.sync.dma_start(out=jc57[:], in_=jc57_d[:])
            wsh1 = cp.tile([114, 128], BF)
            nc.sync.dma_start(out=wsh1[:], in_=wsh1_d[:])
            wsh2 = cp.tile([57, 128], BF)
            nc.sync.dma_start(out=wsh2[:], in_=wsh2_d[:])
            wg = cp.tile([128, 9 * 128], BF)
            nc.sync.dma_start(out=wg[:], in_=wg_d[:])
            wb = cp.tile([128, 9 * 128], BF)
            nc.sync.dma_start(out=wb[:], in_=wb_d[:])
            wcg = cp.tile([128, 4 * 1152], BF)
            nc.sync.dma_start(out=wcg[:], in_=wcg_d[:])
            wcb = cp.tile([128, 4 * 1152], BF)
            nc.sync.dma_start(out=wcb[:], in_=wcb_d[:])
            ag1 = cp.tile([114, 128], BF)
            ag2 = cp.tile([57, 128], BF)
            ab1 = cp.tile([114, 128], BF)
            ab2 = cp.tile([57, 128], BF)
            sums_x = cp.tile([C, NBLK], F32)
            sums_q = cp.tile([C, NBLK], F32)
            stats2 = cp.tile([C, 2], F32)
            stats_g = cp.tile([C, 2], F32)
            m_t = cp.tile([C, 1], F32)
            e_t = cp.tile([C, 1], F32)
            nvar = cp.tile([C, 1], F32)
            varp = cp.tile([C, 1], F32)
            sqv = cp.tile([C, 1], F32)
            rstd = cp.tile([C, 1], F32)
            nmr = cp.tile([C, 1], F32)
            # persistent actv double-buffer (3D: [C, BLK+2, WP])
            actv_a = cp.tile([C, BLK + 2, WP], BF)
            actv_b = cp.tile([C, BLK + 2, WP], BF)
            actv_bufs = [actv_a, actv_b]
            # zero the padding columns once
            for ab in actv_bufs:
                nc.vector.memset(ab[:, :, 0:1], 0.0)
                nc.vector.memset(ab[:, :, WP - 1:WP], 0.0)
            # one-hot quarters, double-buffered by quarter parity so the
            # next quarter's set builds while PE consumes the current one
            s1a = cp.tile([114, QROWS + 2, WP], BF)
            s2a = cp.tile([57, QROWS + 2, WP], BF)
            s1b = cp.tile([114, QROWS + 2, WP], BF)
            s2b = cp.tile([57, QROWS + 2, WP], BF)
            s_sets = [(s1a, s2a), (s1b, s2b)]

            def build_s(q):
                s1t, s2t = s_sets[q % 2]
                rep1 = rp.tile([114, QROWS + 2, WP], I8, tag="rep",
                               name=f"rep1_{q}")
                for g in range(6):
                    dy, dx = OFFS9[g]
                    base = (QROWS * q + 1 + dy) * WP + dx + 1
                    nc.scalar.dma_start(
                        out=rep1[g * L:(g + 1) * L, :, :],
                        in_=lab_d[base:base + SQP].partition_broadcast(L))
                nc.vector.tensor_scalar(s1t[:], rep1[:], jc114[:], None,
                                        AL.is_equal)
                rep2 = rp.tile([57, QROWS + 2, WP], I8, tag="rep",
                               name=f"rep2_{q}")
                for g in range(6, 9):
                    dy, dx = OFFS9[g]
                    base = (QROWS * q + 1 + dy) * WP + dx + 1
                    nc.scalar.dma_start(
                        out=rep2[(g - 6) * L:(g - 5) * L, :, :],
                        in_=lab_d[base:base + SQP].partition_broadcast(L))
                nc.vector.tensor_scalar(s2t[:], rep2[:], jc57[:], None,
                                        AL.is_equal)

            # ---- phase M: mu + A tables ----
            with tc.tile_pool(name="muws", bufs=2) as muws, \
                 tc.tile_pool(name="mu1", bufs=1) as mu1, \
                 tc.tile_pool(name="mups", bufs=1, space="PSUM") as mups:
                sct = mu1.tile([128, 4 * L], BF)
                nc.sync.dma_start(out=sct[:], in_=sct_d[:])
                bfct = mu1.tile([128, 4 * L], F32)
                nc.sync.dma_start(out=bfct[:], in_=bfct_d[:])
                mu_ps = [mups.tile([128, L], F32, tag=f"mups{m}", name=f"mups{m}")
                         for m in range(4)]
                # host layout: [dp, (dc, j, mc, o)] - each dc chunk contiguous
                wfc_sb = mu1.tile([128, 304 * 128], BF)
                CHK = 76 * 128
                for dc in range(4):
                    nc.sync.dma_start(out=wfc_sb[:, dc * CHK:(dc + 1) * CHK],
                                      in_=wfc_d[:, dc * CHK:(dc + 1) * CHK])
                for j in range(L):
                    for mc in range(4):
                        for dc in range(4):
                            nc.tensor.matmul(
                                mu_ps[mc][:, j:j + 1],
                                lhsT=wfc_sb[:, dc * CHK + (j * 4 + mc) * 128:
                                            dc * CHK + (j * 4 + mc + 1) * 128],
                                rhs=sct[:, dc * L + j:dc * L + j + 1],
                                start=(dc == 0), stop=(dc == 3))
                mu_sb = mu1.tile([128, 4 * L], BF)
                for mc in range(4):
                    mtmp = muws.tile([128, L], F32, tag="mtmp", name=f"mtmp{mc}")
                    nc.vector.tensor_add(mtmp[:], mu_ps[mc][:],
                                         bfct[:, mc * L:(mc + 1) * L])
                    nc.scalar.activation(mu_sb[:, mc * L:(mc + 1) * L], mtmp[:],
                                         AF.Relu)
                if DBG and mc == 3:
                    nc.sync.dma_start(out=dbg["d_mu"][:], in_=mu_sb[:])
                # A tables
                for tbl, (wsb, a1, a2) in enumerate(
                        [(wcg, ag1, ag2), (wcb, ab1, ab2)]):
                    for g in range(9):
                        aps = mups.tile([L, 128], F32, tag="aps", name=f"aps{tbl}{g}")
                        for mc in range(4):
                            nc.tensor.matmul(
                                aps[:],
                                lhsT=mu_sb[:, mc * L:(mc + 1) * L],
                                rhs=wsb[:, mc * 1152 + g * 128:mc * 1152 + (g + 1) * 128],
                                start=(mc == 0), stop=(mc == 3))
                        atmp = muws.tile([L, 128], BF, tag="atmp", name=f"atmp{tbl}{g}")
                        nc.scalar.copy(atmp[:], aps[:])
                        dst = (a1 if g < 6 else a2)
                        gg = g if g < 6 else g - 6
                        nc.sync.dma_start(out=dst[gg * L:(gg + 1) * L, :],
                                          in_=atmp[:])
                if DBG:
                    nc.sync.dma_start(out=dbg["d_ag1"][:], in_=ag1[:])

            # ---- main pools ----
            with tc.tile_pool(name="xp", bufs=2) as xp, \
                 tc.tile_pool(name="np_", bufs=2) as np_, \
                 tc.tile_pool(name="ob", bufs=2) as ob, \
                 tc.tile_pool(name="rp", bufs=1) as rp, \
                 tc.tile_pool(name="gsp", bufs=6) as gsp, \
                 tc.tile_pool(name="ps", bufs=2, space="PSUM") as psp, \
                 tc.tile_pool(name="psgb", bufs=4, space="PSUM") as psgb:

                build_s(0)
                build_s(1)
                # ---- phase A: BN partial sums ----
                for ci in range(NBLK):
                    xb = xp.tile([C, BLK * W], F32, tag="x", name=f"xA{ci}")
                    nc.sync.dma_start(out=xb[:],
                                      in_=x_d[:, ci * BLK * W:(ci + 1) * BLK * W])
                    nb = np_.tile([C, BLK * W], BF, tag="n", name=f"nA{ci}")
                    nc.sync.dma_start(
                        out=nb[:],
                        in_=noi_d[ci * BLK * W:(ci + 1) * BLK * W]
                        .partition_broadcast(C))
                    nc.vector.scalar_tensor_tensor(
                        out=xb[:], in0=nb[:], scalar=nv[:], in1=xb[:],
                        op0=AL.mult, op1=AL.add,
                        accum_out=sums_x[:, ci:ci + 1])
                    nc.scalar.activation(xb[:], xb[:], AF.Square,
                                         accum_out=sums_q[:, ci:ci + 1])
                nc.vector.tensor_reduce(out=stats2[:, 0:1], in_=sums_x[:],
                                        axis=mybir.AxisListType.X, op=AL.add)
                nc.vector.tensor_reduce(out=stats2[:, 1:2], in_=sums_q[:],
                                        axis=mybir.AxisListType.X, op=AL.add)
                stat_in = dramp.tile([C, 2], F32)
                stat_out = dramp.tile([C, 2], F32)
                nc.sync.dma_start(out=stat_in[:], in_=stats2[:])
                nc.gpsimd.collective_compute(
                    "AllReduce", AL.add, replica_groups=[list(range(NCORES))],
                    ins=[stat_in.opt()], outs=[stat_out.opt()])
                nc.sync.dma_start(out=stats_g[:], in_=stat_out[:])
                nc.vector.tensor_scalar_mul(m_t[:], stats_g[:, 0:1], 1.0 / NTOT)
                nc.vector.tensor_scalar_mul(e_t[:], stats_g[:, 1:2], 1.0 / NTOT)
                # nvar = m^2 - E[x^2];  varp = -nvar + EPS = var + EPS
                nc.vector.scalar_tensor_tensor(
                    out=nvar[:], in0=m_t[:], scalar=m_t[:], in1=e_t[:],
                    op0=AL.mult, op1=AL.subtract)
                nc.vector.tensor_scalar(varp[:], nvar[:], -1.0, EPS,
                                        AL.mult, AL.add)
                nc.scalar.activation(sqv[:], varp[:], AF.Sqrt)
                nc.vector.reciprocal(rstd[:], sqv[:])
                nc.vector.scalar_tensor_tensor(
                    out=nmr[:], in0=m_t[:], scalar=-1.0, in1=rstd[:],
                    op0=AL.mult, op1=AL.mult)
                if DBG:
                    nc.sync.dma_start(out=dbg["d_stats"][:], in_=stats_g[:])
                    rn = cp.tile([C, 2], F32)
                    nc.vector.tensor_copy(rn[:, 0:1], rstd[:])
                    nc.vector.tensor_copy(rn[:, 1:2], nmr[:])
                    nc.sync.dma_start(out=dbg["d_rn"][:], in_=rn[:])

                # ---- main pass ----
                for kb in range(NBLK):
                    q = kb // (QROWS // BLK)
                    r0 = kb * BLK
                    if kb % (QROWS // BLK) == 0 and q >= 2:
                        build_s(q)
                        if DBG and q == 0:
                            nc.sync.dma_start(out=dbg["d_s1"][:],
                                              in_=s1t[:].rearrange("p a b -> p (a b)"))

                    s1, s2 = s_sets[q % 2]
                    xb = xp.tile([C, BLK * W], F32, tag="x", name=f"xM{kb}")
                    nc.sync.dma_start(out=xb[:],
                                      in_=x_d[:, r0 * W:(r0 + BLK) * W])
                    nb = np_.tile([C, BLK * W], BF, tag="n", name=f"nM{kb}")
                    nc.sync.dma_start(
                        out=nb[:],
                        in_=noi_d[r0 * W:(r0 + BLK) * W].partition_broadcast(C))
                    nc.vector.scalar_tensor_tensor(
                        out=xb[:], in0=nb[:], scalar=nv[:], in1=xb[:],
                        op0=AL.mult, op1=AL.add)
                    # norm in place on the DVE (ACT queue stays stats-free)
                    nc.vector.tensor_scalar(xb[:], xb[:], rstd[:], nmr[:],
                                            AL.mult, AL.add)
                    norm = xb

                    # actv rows ar in [r0-1, r0+BLK+1)
                    actv = actv_bufs[kb % 2]
                    for ir in range(BLK + 2):
                        ar = r0 - 1 + ir
                        pa = psp.tile([C, WP], F32, tag="pa", name=f"pa{kb}_{ir}")
                        nc.tensor.matmul(pa[:], lhsT=wsh1[:],
                                         rhs=_s_rhs1(s1, ar, q),
                                         start=True, stop=False)
                        nc.tensor.matmul(pa[:], lhsT=wsh2[:],
                                         rhs=_s_rhs1(s2, ar, q),
                                         start=False, stop=True)
                        nc.scalar.activation(actv[:, ir, 1:W + 1],
                                             pa[:, 1:W + 1], AF.Relu,
                                             bias=bshm[:, ar + 1:ar + 2])

                    if DBG and kb == 0:
                        nc.sync.dma_start(out=dbg["d_actv"][:],
                                          in_=actv[:].rearrange("p a b -> p (a b)"))
                        nc.sync.dma_start(out=dbg["d_norm"][:], in_=norm[:])
                    # gamma/beta: weight-stationary over the block's 4 row
                    # pairs (1 LDWEIGHTS per 4 matmuls), staged to SBUF bf16
                    # so PE never waits on the BN stats / epilogue.
                    NP2 = BLK // 2
                    gstage = gsp.tile([C, BLK * W], BF, tag="gs", name=f"gs{kb}")
                    bstage = gsp.tile([C, BLK * W], BF, tag="bs", name=f"bs{kb}")
                    for pst_w, a1t, a2t, stage in ((wg, ag1, ag2, gstage),
                                                   (wb, ab1, ab2, bstage)):
                        pps = [psgb.tile([C, 2 * W], F32, tag="pgb",
                                        name=f"p{kb}_{id(stage) % 97}_{i}")
                               for i in range(NP2)]
                        for g, (dy, dx) in enumerate(OFFS9):
                            for i in range(NP2):
                                nc.tensor.matmul(
                                    pps[i][:],
                                    lhsT=pst_w[:, g * 128:(g + 1) * 128],
                                    rhs=actv[:, 2 * i + 1 + dy:2 * i + 3 + dy,
                                             1 + dx:W + 1 + dx],
                                    start=(g == 0), stop=False)
                        for i in range(NP2):
                            nc.tensor.matmul(pps[i][:], lhsT=a1t[:],
                                             rhs=_s_rhs2(s1, r0 + 2 * i, q),
                                             start=False, stop=False)
                        for i in range(NP2):
                            nc.tensor.matmul(pps[i][:], lhsT=a2t[:],
                                             rhs=_s_rhs2(s2, r0 + 2 * i, q),
                                             start=False, stop=True)
                        for i in range(NP2):
                            nc.scalar.copy(stage[:, 2 * i * W:(2 * i + 2) * W],
                                           pps[i][:])
                    if DBG and kb == 0:
                        nc.sync.dma_start(out=dbg["d_pg"][:],
                                          in_=gstage[:, 0:2 * W])
                        nc.sync.dma_start(out=dbg["d_pb"][:],
                                          in_=bstage[:, 0:2 * W])
                    ot = ob.tile([C, BLK * W], F32, tag="ot", name=f"ot{kb}")
                    nc.vector.scalar_tensor_tensor(
                        out=ot[:], in0=gstage[:], scalar=gbias[:],
                        in1=norm[:], op0=AL.add, op1=AL.mult)
                    nc.vector.scalar_tensor_tensor(
                        out=ot[:], in0=bstage[:], scalar=bbias[:], in1=ot[:],
                        op0=AL.add, op1=AL.add)
                    nc.sync.dma_start(out=out_d[:, r0 * W:(r0 + BLK) * W],
                                       in_=ot[:])
    nc.compile()
    return nc


def _prep_shared(inputs):
    """Host-side weight layout prep (replicated to all cores)."""
    gb = np.asarray(inputs["g_blend"], np.float32).reshape(-1)[0]
    bb = np.asarray(inputs["b_blend"], np.float32).reshape(-1)[0]
    ga = 1.0 / (1.0 + np.exp(-gb))
    ba = 1.0 / (1.0 + np.exp(-bb))
    w_sh = np.asarray(inputs["w_sh"], np.float32)
    w_g = np.asarray(inputs["w_g"], np.float32)
    w_b = np.asarray(inputs["w_b"], np.float32)
    w_cg = np.asarray(inputs["w_cg"], np.float32)
    w_cb = np.asarray(inputs["w_cb"], np.float32)
    Wfc = np.asarray(inputs["Wfc"], np.float32)
    bfc = np.asarray(inputs["bfc"], np.float32)
    b_sh = np.asarray(inputs["b_sh"], np.float32)
    b_g = np.asarray(inputs["b_g"], np.float32)
    b_b = np.asarray(inputs["b_b"], np.float32)
    b_cg = np.asarray(inputs["b_cg"], np.float32)
    b_cb = np.asarray(inputs["b_cb"], np.float32)
    nv = np.asarray(inputs["noise_var"], np.float32)

    sh = {}
    # w_sh [o, j, 3, 3] -> [(dy,dx,j), o] stacked
    wshst = np.ascontiguousarray(
        w_sh.transpose(2, 3, 1, 0).reshape(9 * L, 128)).astype(BF16)
    sh["wsh1"] = np.ascontiguousarray(wshst[:6 * L])
    sh["wsh2"] = np.ascontiguousarray(wshst[6 * L:])
    # w_g/w_b [o, c, 3, 3] -> [c, (g, o)] scaled
    sh["wg"] = np.ascontiguousarray(
        ((1 - ga) * w_g).transpose(1, 2, 3, 0).reshape(128, 9 * 128)).astype(BF16)
    sh["wb"] = np.ascontiguousarray(
        ((1 - ba) * w_b).transpose(1, 2, 3, 0).reshape(128, 9 * 128)).astype(BF16)
    # w_cg/w_cb [o, c(512), 3, 3] -> [128, (cc, g, o)] scaled
    def cvt_cw(wt, scale):
        a = (scale * wt).transpose(1, 2, 3, 0).reshape(512, 9 * 128)
        a = a.reshape(4, 128, 9 * 128).transpose(1, 0, 2).reshape(128, 4 * 1152)
        return np.ascontiguousarray(a).astype(BF16)
    sh["wcg"] = cvt_cw(w_cg, ga)
    sh["wcb"] = cvt_cw(w_cb, ba)
    # Wfc [j, o, d] -> [128(dp), (j, dc, mc)*128 + o]
    f = Wfc.transpose(0, 2, 1).reshape(L, 4, 128, 4, 128)
    f = f.transpose(0, 1, 3, 2, 4)            # [j, dc, mc, dp, o]
    f = f.transpose(3, 1, 0, 2, 4).reshape(128, 304 * 128)  # [dp, dc, j, mc, o]
    sh["wfc"] = np.ascontiguousarray(f).astype(BF16)
    # bfc [j, d] -> bfct [128, (mc, j)]
    bf_t = bfc.T.reshape(4, 128, L).transpose(1, 0, 2).reshape(128, 4 * L)
    sh["bfct"] = np.ascontiguousarray(bf_t).astype(np.float32)
    sh["nv"] = np.ascontiguousarray(nv.reshape(C, 1))
    sh["gbias"] = np.ascontiguousarray(
        (1.0 + ga * b_cg + (1 - ga) * b_g).reshape(C, 1)).astype(np.float32)
    sh["bbias"] = np.ascontiguousarray(
        (ba * b_cb + (1 - ba) * b_b).reshape(C, 1)).astype(np.float32)
    sh["jc114"] = np.tile(np.arange(L, dtype=np.float32), 6)[:, None].copy()
    sh["jc57"] = np.tile(np.arange(L, dtype=np.float32), 3)[:, None].copy()
    sh["_b_sh"] = b_sh
    return sh


def kernel(**inputs):
    if "nc" not in _cache:
        _cache["nc"] = build_nc()
    nc = _cache["nc"]

    x = np.asarray(inputs["x"], np.float32)
    labels = np.asarray(inputs["labels"]).astype(np.int64)
    noise = np.asarray(inputs["noise"], np.float32)
    style = np.asarray(inputs["style_codes"], np.float32)
    sh = _prep_shared(inputs)
    b_sh = sh.pop("_b_sh")

    in_maps = []
    for c in range(NCORES):
        b, half = c // 2, c % 2
        h0 = half * NROWS
        m = dict(sh)
        m["x"] = np.ascontiguousarray(
            x[b, :, h0:h0 + NROWS, :]).reshape(C, NPIX)
        # noise [B, W, H, 1]: added[c,h,w] = noise[b,w,h]*nv[c]
        m["noiseT"] = np.ascontiguousarray(
            noise[b, :, h0:h0 + NROWS, 0].T).reshape(NPIX).astype(BF16)
        # guarded, padded labels (int8, -1 outside image)
        lab = np.full((NROWS + 4, WP), -1, np.int8)
        lo, hi = max(0, h0 - 2), min(H, h0 + NROWS + 2)
        lab[lo - (h0 - 2):hi - (h0 - 2), 1:W + 1] = labels[b, lo:hi, :]
        g = np.full(LABN, -1, np.int8)
        g[1:1 + (NROWS + 4) * WP] = lab.reshape(-1)
        m["labels"] = g
        # style codes transposed [128, (dc, j)]
        sct = style[b].T.reshape(4, 128, L).transpose(1, 0, 2).reshape(128, 4 * L)
        m["sct"] = np.ascontiguousarray(sct).astype(BF16)
        # b_sh masked per actv row (zero outside image)
        rows = h0 + np.arange(-1, NROWS + 1)
        mask = ((rows >= 0) & (rows < H)).astype(np.float32)
        m["bshm"] = np.ascontiguousarray(b_sh[:, None] * mask[None, :])
        in_maps.append(m)

    res = run_bass_kernel_spmd(nc, in_maps, core_ids=list(range(NCORES)),
                               **_cache.get("run_kwargs", {}))
    _cache["last_result"] = res

    out = np.empty((B, C, H, W), np.float32)
    for c in range(NCORES):
        b, half = c // 2, c % 2
        h0 = half * NROWS
        out[b, :, h0:h0 + NROWS, :] = res.results[c]["out"].reshape(C, NROWS, W)
    return out
